# revision 45
# baseline (speedup 1.0000x reference)
"""BysMamba Trainium2 kernel: 8-core SPMD bass/Tile implementation (v7).

Sharding: core c = (batch b = c//4) x (d_inner shard s = c%4, 128 channels).
Replica groups [[0..3],[4..7]] (one per batch). The fp32 residual stream h
(256 x 2048, replicated within each group) lives in SBUF for the whole
kernel.

Each directional mamba pass (12 total; layers 0 and 9 bidirectional) is
split into two L-halves of 1024 columns so each half's collectives overlap
the other half's compute. The scan state crosses the boundary via a saved
[128,16] boundary-column tile used as the per-partition `initial` of the
second half's scans. Half 0 runs all 16 states, launches its gate/out_proj/
ReduceScatter+AllGather, then half 1's state loop runs under those
collectives.

Engine assignment:
  PE:   in_proj with the causal depthwise conv folded into the matmul
        weights (streamed per-layer from DRAM); z-proj (fwd passes only --
        rev reuses the flipped fwd silu(z) tile); x_proj partial; dt_proj;
        per-state identity-matmul accumulation of sum_n g_n into PSUM;
        host-baked diag(d_param) matmul adds d*xi into the same PSUM bank;
        out_proj partial.
  Act:  SiLU(xi), SiLU(z), e=Exp(s+b_dt), delta=Ln(1+e), 32x dA =
        Exp(A_n*delta) halves (A_n host-known immediates), boundary-state
        copies, out-partial fp32->bf16 casts, hrv refresh casts.
  DVE:  u=delta*xi, dBx_n=u*B_n and the scan back-to-back (producer
        queue), 4 of 16 g_n=h_n*C_n, y=(psum)*SiLU(z), h updates
        (hbf=h32+out and h32+=out as independent parallel adds).
  Pool: 12 of 16 g_n mults (Pool cannot touch PSUM; ~2.1us flat per op).
  DMA:  B_n/C_n row broadcasts from the AllReduduced dbc DRAM tile.
  Collectives per half: AllReduce(48x1024 bf16) for x_proj partials;
        ReduceScatter+AllGather (bf16) for out_proj partials.

Per-pass tiles are double-buffered by pass parity so independent passes
(the two directions of a bidirectional layer) overlap.

Front-end: 3x3 patch conv2d folded on the host into 9 gather tables
(emb @ conv2d_w position slices, center + 0.5 + bias folded); device does
indirect-DMA gathers L-sharded across the group + bf16 AllGather.
Back-end: lm_head computed over the full L on every core (SPMD cannot bake
per-core token offsets); host takes each core's slice.
"""
import sys
import os

for _p in ("/opt/trn_rl_repo", "/root/.axon_site/_ro/trn_rl_repo"):
    if os.path.isdir(_p) and _p not in sys.path:
        sys.path.insert(0, _p)

import numpy as np
import ml_dtypes

import concourse.bass as bass
import concourse.tile as tile
from concourse import mybir
from concourse.bass_utils import run_bass_kernel_spmd

BF = ml_dtypes.bfloat16
F32 = mybir.dt.float32
BF16 = mybir.dt.bfloat16
I32 = mybir.dt.int32

B = 2
L = 2048
DIM = 256
DIN = 512
DSH = 128
NST = 16
DTR = 16
VOCAB = 474
NM = 10
LPAD = 3
LT = L + LPAD
LSH = 512
NCORES = 8
GROUPS = [[0, 1, 2, 3], [4, 5, 6, 7]]

N_LAYERS = NM          # bring-up override
DEBUG_DUMP_H = False   # adds an "hdump" output with the final residual h

_prog_cache = {}


def _split_excess_waits(nc, max_waits=1):
    """walrus here rejects >1 sync-wait per instruction; split the excess
    onto same-engine NoOps placed immediately before."""
    n = 0
    for fn in nc.m.functions:
        for blk in fn.blocks:
            out = []
            changed = False
            for inst in blk.instructions:
                si = inst.sync_info
                waits = list(si.on_wait) if si is not None and si.on_wait else []
                if len(waits) > max_waits:
                    extra = waits[:-max_waits]
                    si.on_wait = waits[-max_waits:]
                    for i in range(0, len(extra), max_waits):
                        out.append(mybir.InstNoOp(
                            name=f"{inst.name}-wsplit-{i}",
                            engine=inst.engine, ins=[], outs=[],
                            sync_info=mybir.SyncInfo(
                                on_wait=extra[i:i + max_waits], on_update=[]),
                        ))
                        n += 1
                    changed = True
                out.append(inst)
            if changed:
                blk.instructions = out
    return n


def _bcast_row_ap(tile_ap, row, width):
    """AP reading one row replicated across 128 partitions."""
    r = tile_ap[row:row + 1, :]
    return bass.AP(tensor=r.tensor, offset=r.offset, ap=[[0, 128], [1, width]])


def _build_program(a_scales, n_layers, dump_h):
    AOP = mybir.AluOpType
    AF = mybir.ActivationFunctionType

    nc = bass.Bass(num_devices=NCORES)

    def par(name, shape, dt):
        return nc.declare_dram_parameter(name, list(shape), dt, isOutput=False)

    t9 = par("t9", (9 * VOCAB, DIM), F32)
    idxp = par("idxp", (128, 36), I32)
    wconv = par("wconv", (128, NM * 8 * 128), BF16)
    wz = par("wz", (128, NM * 2 * 128), BF16)
    wx = par("wx", (128, NM * 48), BF16)
    wdt = par("wdt", (16, NM * 128), BF16)
    wout = par("wout", (128, NM * 256), BF16)
    wdiag = par("wdiag", (128, NM * 128), BF16)
    lmh = par("lmh", (128, 2 * VOCAB), BF16)
    bdtp = par("bdt", (128, NM), F32)
    cbp = par("cb", (128, NM), F32)
    b9p = par("b9", (128, 2), F32)
    identb = par("identb", (128, 128), BF16)
    identf = par("identf", (128, 128), F32)

    logits = nc.declare_dram_parameter("logits", [VOCAB, L], F32, isOutput=True)
    hdump = None
    if dump_h:
        hdump = nc.declare_dram_parameter("hdump", [2, 128, LT], F32, isOutput=True)

    import contextlib
    with tile.TileContext(nc) as tc, contextlib.ExitStack() as ctx:
        persist = ctx.enter_context(tc.tile_pool(name="persist", bufs=1))
        ps = ctx.enter_context(tc.tile_pool(name="ps", bufs=2, space="PSUM"))
        bc = ctx.enter_context(tc.tile_pool(name="bc", bufs=4))
        wk = ctx.enter_context(tc.tile_pool(name="wk", bufs=3))
        ws = ctx.enter_context(tc.tile_pool(name="ws", bufs=2))
        fe = ctx.enter_context(tc.tile_pool(name="fe", bufs=3))
        dram = ctx.enter_context(tc.tile_pool(name="dram", bufs=2, space="DRAM"))

        def ld(param, shape, dt, tag):
            t = persist.tile(list(shape), dt, tag=tag, name=tag)
            nc.sync.dma_start(out=t[:], in_=param[:])
            return t

        wz_s = ld(wz, (128, NM * 2 * 128), BF16, "wz_s")
        wx_s = ld(wx, (128, NM * 48), BF16, "wx_s")
        wdt_s = ld(wdt, (16, NM * 128), BF16, "wdt_s")
        wout_s = ld(wout, (128, NM * 256), BF16, "wout_s")
        wdiag_s = ld(wdiag, (128, NM * 128), BF16, "wdiag_s")
        lmh_s = ld(lmh, (128, 2 * VOCAB), BF16, "lmh_s")
        bdt_s = ld(bdtp, (128, NM), F32, "bdt_s")
        cb_s = ld(cbp, (128, NM), F32, "cb_s")
        b9_s = ld(b9p, (128, 2), F32, "b9_s")
        idb_s = ld(identb, (128, 128), BF16, "idb_s")
        idf_s = ld(identf, (128, 128), F32, "idf_s")
        idx_s = ld(idxp, (128, 36), I32, "idx_s")

        h32 = [persist.tile([128, LT], F32, tag=f"h32_{k}", name=f"h32_{k}") for k in range(2)]
        hbf = [persist.tile([128, LT], BF16, tag=f"hbf_{k}", name=f"hbf_{k}") for k in range(2)]
        hrv = [persist.tile([128, LT], BF16, tag=f"hrv_{k}", name=f"hrv_{k}") for k in range(2)]
        for k in range(2):
            nc.vector.memset(h32[k][:], 0.0)
            nc.vector.memset(hbf[k][:], 0.0)
            nc.vector.memset(hrv[k][:], 0.0)

        # per-pass tiles, double-buffered by pass parity
        xi_t = [persist.tile([128, L], BF16, tag=f"xi_{p}", name=f"xi_{p}") for p in range(2)]
        sz_t = [persist.tile([128, L], BF16, tag=f"sz_{p}", name=f"sz_{p}") for p in range(2)]
        u_t = [persist.tile([128, L], BF16, tag=f"u_{p}", name=f"u_{p}") for p in range(2)]
        dl_t = [persist.tile([128, L], BF16, tag=f"dl_{p}", name=f"dl_{p}") for p in range(2)]
        dbc_t = [persist.tile([48, L], BF16, tag=f"dbc_{p}", name=f"dbc_{p}") for p in range(2)]
        y2_t = [persist.tile([128, L], BF16, tag=f"y2_{p}", name=f"y2_{p}") for p in range(2)]
        outp = [persist.tile([128, L], BF16, tag=f"outp_{k}", name=f"outp_{k}") for k in range(2)]
        outf = [persist.tile([128, L], BF16, tag=f"outf_{k}", name=f"outf_{k}") for k in range(2)]
        h0loc = [persist.tile([128, LSH], BF16, tag=f"h0loc_{k}", name=f"h0loc_{k}") for k in range(2)]

        # ---- front-end -----------------------------------------------------
        ptt = ps.tile([128, 2048], F32, tag="ps", name="ps")
        for tau in range(4):
            acc = fe.tile([128, DIM], F32, tag="feacc", name="feacc")
            for j in range(9):
                g = fe.tile([128, DIM], F32, tag="feg", name="feg")
                nc.gpsimd.indirect_dma_start(
                    out=g[:], out_offset=None, in_=t9[:],
                    in_offset=bass.IndirectOffsetOnAxis(
                        ap=idx_s[:, tau * 9 + j: tau * 9 + j + 1], axis=0),
                )
                if j == 0:
                    nc.vector.tensor_copy(out=acc[:], in_=g[:])
                else:
                    nc.vector.tensor_tensor(out=acc[:], in0=acc[:], in1=g[:],
                                             op=AOP.add)
            for dh in range(2):
                blk = tau * 2 + dh
                nc.tensor.transpose(
                    out=ptt[:, blk * 128:(blk + 1) * 128],
                    in_=acc[:, dh * 128:(dh + 1) * 128],
                    identity=idf_s[:])
                nc.vector.tensor_scalar(
                    out=h0loc[dh][:, tau * 128:(tau + 1) * 128],
                    in0=ptt[:, blk * 128:(blk + 1) * 128],
                    scalar1=b9_s[:, dh:dh + 1], scalar2=None, op0=AOP.add)

        agi = dram.tile([2, 128, LSH], BF16, tag="agi", name="agi")
        ago = dram.tile([4, 2, 128, LSH], BF16, tag="ago", name="ago")
        for k in range(2):
            nc.sync.dma_start(out=agi[k], in_=h0loc[k][:])
        nc.gpsimd.collective_compute(
            "AllGather", AOP.bypass, replica_groups=GROUPS,
            ins=[agi.opt()], outs=[ago.opt()])
        for g in range(4):
            for k in range(2):
                nc.sync.dma_start(
                    out=hbf[k][:, LPAD + g * LSH: LPAD + (g + 1) * LSH],
                    in_=ago[g, k])
        for k in range(2):
            nc.scalar.activation(out=h32[k][:, LPAD:], in_=hbf[k][:, LPAD:],
                                 func=AF.Copy, scale=1.0)

        # ---- one directional mamba pass ------------------------------------
        HL = L // 2  # half length for collective/compute pipelining

        def mamba_pass(l, hb, out_dst, p, sz_rev=None):
            co = l * 8 * 128
            wc_l = ws.tile([128, 8 * 128], BF16, tag="wc_l", name="wc_l")
            nc.sync.dma_start(out=wc_l[:], in_=wconv[:, co:co + 8 * 128])

            cin = [dram.tile([48, HL], BF16, tag=f"cin{q}", name=f"cin{q}")
                   for q in range(2)]
            cout = [dram.tile([48, HL], BF16, tag=f"cout{q}", name=f"cout{q}")
                    for q in range(2)]

            pxc = ps.tile([128, 2048], F32, tag="ps", name="ps")
            pz = None
            if sz_rev is None:
                pz = ps.tile([128, 2048], F32, tag="ps", name="ps")
            pxp = ps.tile([128, 2048], F32, tag="ps", name="ps")
            pdt = ps.tile([128, 2048], F32, tag="ps", name="ps")

            # ---- front chain helpers (emission order matters: each engine
            # queue is in-order, so ops are emitted in expected-run order) --
            def front_head(q):
                # in_proj(conv) -> SiLU -> [z-proj] -> x_proj -> AllReduce
                c0 = q * HL
                for m in range(2):          # 512-col chunks within the half
                    cc = c0 + m * 512
                    for kt in range(2):
                        for j in range(4):
                            lt = wc_l[:, (j * 2 + kt) * 128:
                                      (j * 2 + kt) * 128 + 128]
                            nc.tensor.matmul(
                                out=pxc[:, cc:cc + 512],
                                lhsT=lt,
                                rhs=hb[kt][:, cc + j: cc + j + 512],
                                start=(kt == 0 and j == 0),
                                stop=(kt == 1 and j == 3))
                nc.scalar.activation(out=xi_t[p][:, c0:c0 + HL],
                                     in_=pxc[:, c0:c0 + HL], func=AF.Silu,
                                     bias=cb_s[:, l:l + 1], scale=1.0)

                if sz_rev is None:
                    for m in range(2):
                        cc = c0 + m * 512
                        for kt in range(2):
                            lt = wz_s[:, (l * 2 + kt) * 128:
                                      (l * 2 + kt) * 128 + 128]
                            nc.tensor.matmul(
                                out=pz[:, cc:cc + 512],
                                lhsT=lt,
                                rhs=hb[kt][:, LPAD + cc: LPAD + cc + 512],
                                start=(kt == 0), stop=(kt == 1))
                    nc.scalar.activation(out=sz_t[p][:, c0:c0 + HL],
                                         in_=pz[:, c0:c0 + HL], func=AF.Silu,
                                         scale=1.0)

                for m in range(2):
                    cc = c0 + m * 512
                    nc.tensor.matmul(
                        out=pxp[:48, cc:cc + 512],
                        lhsT=wx_s[:, l * 48:(l + 1) * 48],
                        rhs=xi_t[p][:, cc:cc + 512],
                        start=True, stop=True)
                dbc_part = wk.tile([48, HL], BF16, tag="dbc_part",
                                   name="dbc_part", bufs=2)
                nc.scalar.activation(out=dbc_part[:], in_=pxp[:48, c0:c0 + HL],
                                     func=AF.Copy, scale=1.0)
                nc.sync.dma_start(out=cin[q][:], in_=dbc_part[:])
                nc.gpsimd.collective_compute(
                    "AllReduce", AOP.add, replica_groups=GROUPS,
                    ins=[cin[q].opt()], outs=[cout[q].opt()])
                nc.sync.dma_start(out=dbc_t[p][:, c0:c0 + HL], in_=cout[q][:])

            def front_tail(q):
                # dt_proj -> softplus -> u, after this half's AllReduce
                c0 = q * HL
                for m in range(2):
                    cc = c0 + m * 512
                    nc.tensor.matmul(
                        out=pdt[:, cc:cc + 512],
                        lhsT=wdt_s[:, l * 128:(l + 1) * 128],
                        rhs=dbc_t[p][:16, cc:cc + 512],
                        start=True, stop=True)
                e_b = wk.tile([128, HL], BF16, tag="e_b", name="e_b", bufs=2)
                nc.scalar.activation(out=e_b[:], in_=pdt[:, c0:c0 + HL],
                                     func=AF.Exp,
                                     bias=bdt_s[:, l:l + 1], scale=1.0)
                nc.scalar.activation(out=dl_t[p][:, c0:c0 + HL], in_=e_b[:],
                                     func=AF.Ln, bias=1.0, scale=1.0)
                nc.vector.tensor_tensor(out=u_t[p][:, c0:c0 + HL],
                                        in0=dl_t[p][:, c0:c0 + HL],
                                        in1=xi_t[p][:, c0:c0 + HL],
                                        op=AOP.mult)

            front_head(0)
            front_tail(0)
            front_head(1)

            # ---- state loop + per-half out pipeline ------------------------
            # Half 0 runs all 16 states first (saving each boundary state),
            # then its gate/out_proj/reduce overlaps half 1's state loop.
            py = ps.tile([128, 2048], F32, tag="ps", name="ps")
            hstate = wk.tile([128, NST], BF16, tag="hstate", name="hstate",
                             bufs=2)
            oin = [dram.tile([2, 128, HL], BF16, tag=f"oin{q}", name=f"oin{q}")
                   for q in range(2)]
            ors = [dram.tile([2 * 128 * HL // 4], BF16, tag=f"ors{q}",
                             name=f"ors{q}") for q in range(2)]
            oout = [dram.tile([2, 128, HL], BF16, tag=f"oout{q}",
                              name=f"oout{q}") for q in range(2)]
            for q in range(2):
                if q == 1:
                    front_tail(1)
                c0 = q * HL
                # d_param * xi accumulated first via host-baked diag matmul
                for m in range(2):
                    cc = c0 + m * 512
                    nc.tensor.matmul(
                        out=py[:, cc:cc + 512],
                        lhsT=wdiag_s[:, l * 128:(l + 1) * 128],
                        rhs=xi_t[p][:, cc:cc + 512],
                        start=True, stop=False)
                for n in range(NST):
                    # dbx+scan back-to-back on DVE (producer queue), all g_n
                    # on Pool (consumer queue): no cross-engine queue stalls
                    bbc = bc.tile([128, HL], BF16, tag="bbc", name="bbc")
                    nc.sync.dma_start(out=bbc[:],
                                      in_=_bcast_row_ap(cout[q], 16 + n, HL))
                    cbc = bc.tile([128, HL], BF16, tag="cbc", name="cbc")
                    nc.sync.dma_start(out=cbc[:],
                                      in_=_bcast_row_ap(cout[q], 32 + n, HL))
                    da = wk.tile([128, HL], BF16, tag="da", name="da", bufs=6)
                    nc.scalar.activation(out=da[:], in_=dl_t[p][:, c0:c0 + HL],
                                         func=AF.Exp,
                                         scale=float(a_scales[l][n]))
                    dbx = wk.tile([128, HL], BF16, tag="dbx", name="dbx",
                                  bufs=4)
                    nc.vector.tensor_tensor(out=dbx[:],
                                            in0=u_t[p][:, c0:c0 + HL],
                                            in1=bbc[:], op=AOP.mult)
                    hn = wk.tile([128, HL], BF16, tag="hn", name="hn", bufs=4)
                    nc.vector.tensor_tensor_scan(
                        out=hn[:], data0=da[:], data1=dbx[:],
                        initial=(0.0 if q == 0 else hstate[:, n:n + 1]),
                        op0=AOP.mult, op1=AOP.add)
                    if q == 0:
                        nc.scalar.activation(out=hstate[:, n:n + 1],
                                             in_=hn[:, HL - 1:HL],
                                             func=AF.Copy, scale=1.0)
                    gn = wk.tile([128, HL], BF16, tag="gn", name="gn", bufs=4)
                    # all-DVE state chain: dbx/scan/gn back-to-back with no
                    # cross-engine hops beats any DVE/Pool work split
                    nc.vector.tensor_tensor(out=gn[:], in0=hn[:], in1=cbc[:],
                                            op=AOP.mult)
                    for m in range(2):
                        cc = c0 + m * 512
                        nc.tensor.matmul(
                            out=py[:, cc:cc + 512],
                            lhsT=idb_s[:],
                            rhs=gn[:, m * 512:(m + 1) * 512],
                            start=False, stop=(n == NST - 1))

                # gate + out_proj + reduce for this half
                if sz_rev is None:
                    sz_ap = sz_t[p][:, c0:c0 + HL]
                else:
                    stop = L - 1 - c0 - HL
                    sz_ap = sz_rev[:, L - 1 - c0: (None if stop < 0 else stop):-1]
                nc.vector.tensor_tensor(out=y2_t[p][:, c0:c0 + HL],
                                        in0=py[:, c0:c0 + HL], in1=sz_ap,
                                        op=AOP.mult)
                po = ps.tile([128, 2048], F32, tag="ps", name="ps")
                for mt in range(2):
                    for m in range(2):
                        cc = c0 + m * 512
                        nc.tensor.matmul(
                            out=po[:, mt * 1024 + m * 512:
                                   mt * 1024 + m * 512 + 512],
                            lhsT=wout_s[:, l * 256 + mt * 128:
                                        l * 256 + mt * 128 + 128],
                            rhs=y2_t[p][:, cc:cc + 512],
                            start=True, stop=True)
                pob = wk.tile([128, L], BF16, tag="pob", name="pob", bufs=2)
                nc.scalar.activation(out=pob[:], in_=po[:], func=AF.Copy,
                                     scale=1.0)
                for mt in range(2):
                    nc.sync.dma_start(out=oin[q][mt],
                                      in_=pob[:, mt * 1024: mt * 1024 + 1024])
                nc.gpsimd.collective_compute(
                    "ReduceScatter", AOP.add, replica_groups=GROUPS,
                    ins=[oin[q].opt()], outs=[ors[q].opt()])
                nc.gpsimd.collective_compute(
                    "AllGather", AOP.bypass, replica_groups=GROUPS,
                    ins=[ors[q].opt()], outs=[oout[q].opt()])
                for mt in range(2):
                    nc.sync.dma_start(out=out_dst[mt][:, c0:c0 + HL],
                                      in_=oout[q][mt])

        def refresh_hrv():
            for k in range(2):
                for q in range(2):
                    c0 = q * HL
                    stop = LT - 1 - c0 - HL
                    nc.scalar.activation(
                        out=hrv[k][:, LPAD + c0: LPAD + c0 + HL],
                        in_=hbf[k][:, LT - 1 - c0: (None if stop < 0 else stop):-1],
                        func=AF.Copy, scale=1.0)

        pidx = 0
        for li in range(min(n_layers, NM)):
            bidir = (li == 0 or li == NM - 1)
            if bidir:
                pf, pr = pidx % 2, (pidx + 1) % 2
                pidx += 2
                mamba_pass(li, hbf, outf, pf)
                refresh_hrv()
                mamba_pass(li, hrv, outp, pr, sz_rev=sz_t[pf])
                for q in range(2):
                    c0 = q * HL
                    stop = L - 1 - c0 - HL
                    rsl = slice(L - 1 - c0, (None if stop < 0 else stop), -1)
                    for k in range(2):
                        tsum = wk.tile([128, HL], BF16, tag="tsum",
                                       name="tsum", bufs=2)
                        nc.vector.tensor_tensor(
                            out=tsum[:], in0=outf[k][:, c0:c0 + HL],
                            in1=outp[k][:, rsl], op=AOP.add)
                        # hbf read of old h32 is emitted first; the in-place
                        # h32 update is ordered after it by the WAR dependency
                        nc.vector.tensor_tensor(
                            out=hbf[k][:, LPAD + c0: LPAD + c0 + HL],
                            in0=h32[k][:, LPAD + c0: LPAD + c0 + HL],
                            in1=tsum[:], op=AOP.add)
                        nc.vector.tensor_tensor(
                            out=h32[k][:, LPAD + c0: LPAD + c0 + HL],
                            in0=h32[k][:, LPAD + c0: LPAD + c0 + HL],
                            in1=tsum[:], op=AOP.add)
            else:
                mamba_pass(li, hbf, outp, pidx % 2)
                pidx += 1
                for q in range(2):
                    c0 = q * HL
                    for k in range(2):
                        nc.vector.tensor_tensor(
                            out=hbf[k][:, LPAD + c0: LPAD + c0 + HL],
                            in0=h32[k][:, LPAD + c0: LPAD + c0 + HL],
                            in1=outp[k][:, c0:c0 + HL], op=AOP.add)
                        nc.vector.tensor_tensor(
                            out=h32[k][:, LPAD + c0: LPAD + c0 + HL],
                            in0=h32[k][:, LPAD + c0: LPAD + c0 + HL],
                            in1=outp[k][:, c0:c0 + HL], op=AOP.add)

        # ---- lm_head over full L (host slices per core) --------------------
        for mt in range(4):
            m0 = mt * 128
            msz = min(128, VOCAB - m0)
            plh = ps.tile([128, 2048], F32, tag="ps", name="ps")
            for nt in range(4):
                for kt in range(2):
                    nc.tensor.matmul(
                        out=plh[:msz, nt * 512:(nt + 1) * 512],
                        lhsT=lmh_s[:, kt * VOCAB + m0: kt * VOCAB + m0 + msz],
                        rhs=hbf[kt][:, LPAD + nt * 512: LPAD + nt * 512 + 512],
                        start=(kt == 0), stop=(kt == 1))
            lout = wk.tile([128, L], F32, tag="lout", name="lout", bufs=1)
            nc.scalar.activation(out=lout[:msz, :], in_=plh[:msz, :],
                                 func=AF.Copy, scale=1.0)
            nc.sync.dma_start(out=logits[m0:m0 + msz, :], in_=lout[:msz, :])

        if hdump is not None:
            for k in range(2):
                nc.sync.dma_start(out=hdump[k], in_=h32[k][:])

    return nc


# --------------------------------------------------------------------------
def _host_prep(inputs):
    f = np.float32
    x = np.asarray(inputs["x"]).astype(np.int64).reshape(B, L, 9)
    emb = np.asarray(inputs["emb"], f)
    c2w = np.asarray(inputs["conv2d_w"], f)
    c2b = np.asarray(inputs["conv2d_b"], f)
    w_in = np.asarray(inputs["w_in"], f)
    conv_w = np.asarray(inputs["conv_w"], f)
    conv_b = np.asarray(inputs["conv_b"], f)
    w_x = np.asarray(inputs["w_x"], f)
    w_dt = np.asarray(inputs["w_dt"], f)
    b_dt = np.asarray(inputs["b_dt"], f)
    a_log = np.asarray(inputs["a_log"], f)
    d_param = np.asarray(inputs["d_param"], f)
    w_out = np.asarray(inputs["w_out"], f)
    lm_head = np.asarray(inputs["lm_head"], f)

    # 9 gather tables: position (i,jj) j=3i+jj; T9[j] = 0.5*emb@c2w[:,:,i,jj].T
    t9 = np.empty((9, VOCAB, DIM), f)
    for j in range(9):
        i, jj = divmod(j, 3)
        t9[j] = 0.5 * (emb @ c2w[:, :, i, jj].T)
    t9[4] += 0.5 * emb
    t9f = np.ascontiguousarray(t9.reshape(9 * VOCAB, DIM))
    b9 = 0.5 * c2b  # (256,)

    a_scales = [[float(-np.exp(a_log[l, 0, n])) for n in range(NST)]
                for l in range(NM)]

    per_core = []
    for c in range(NCORES):
        b, s = divmod(c, 4)
        ds = slice(128 * s, 128 * s + 128)
        dglob = np.arange(128 * s, 128 * s + 128)

        # indices for this core's token slice, flattened into t9f rows
        tok = np.arange(LSH * s, LSH * (s + 1))
        idx = (np.arange(9)[None, :] * VOCAB + x[b][tok]).astype(np.int32)  # (512, 9)
        idxp = np.zeros((128, 36), np.int32)
        for tau in range(4):
            idxp[:, tau * 9:(tau + 1) * 9] = idx[tau * 128:(tau + 1) * 128]

        wconv = np.zeros((128, NM * 8 * 128), BF)
        wzv = np.zeros((128, NM * 2 * 128), BF)
        wxv = np.zeros((128, NM * 48), BF)
        wdtv = np.zeros((16, NM * 128), BF)
        woutv = np.zeros((128, NM * 256), BF)
        wdiagv = np.zeros((128, NM * 128), BF)
        for l in range(NM):
            wi = w_in[l][:DIN][ds]          # (128, 256) xi rows
            wzr = w_in[l][DIN:][ds]         # (128, 256) z rows
            cw = conv_w[l][ds]              # (128, 4)
            for j in range(4):
                for kt in range(2):
                    blkc = (l * 8 + j * 2 + kt) * 128
                    # lhsT[kk, d] = cw[d, j] * wi[d, kt*128+kk]
                    wconv[:, blkc:blkc + 128] = (cw[:, j][None, :]
                                                 * wi[:, kt * 128:kt * 128 + 128].T)
            for kt in range(2):
                blkz = (l * 2 + kt) * 128
                wzv[:, blkz:blkz + 128] = wzr[:, kt * 128:kt * 128 + 128].T
            wxv[:, l * 48:(l + 1) * 48] = w_x[l][:, dglob].T  # [d_shard, 48]
            wdtv[:, l * 128:(l + 1) * 128] = w_dt[l][dglob].T  # [16, 128]
            sc = 0.5 if (l == 0 or l == NM - 1) else 1.0
            woutv[:, l * 256:(l + 1) * 256] = sc * w_out[l][:, dglob].T
            wdiagv[:, l * 128:(l + 1) * 128] = np.diag(d_param[l][ds])

        lmhv = np.zeros((128, 2 * VOCAB), BF)
        for kt in range(2):
            lmhv[:, kt * VOCAB:(kt + 1) * VOCAB] = lm_head[:, kt * 128:(kt + 1) * 128].T

        per_core.append({
            "t9": t9f,
            "idxp": idxp,
            "wconv": wconv, "wz": wzv, "wx": wxv, "wdt": wdtv, "wout": woutv,
            "wdiag": wdiagv,
            "lmh": lmhv,
            "bdt": np.ascontiguousarray(b_dt[:, ds].T.astype(f)
                                        if b_dt.ndim == 2 else b_dt),
            "cb": np.ascontiguousarray(conv_b[:, ds].T.astype(f)),
            "b9": np.ascontiguousarray(b9.reshape(2, 128).T.astype(f)),
            "identb": np.eye(128, dtype=BF),
            "identf": np.eye(128, dtype=f),
        })
    return per_core, a_scales


TRACE = False
LAST_EXEC_NS = None
LAST_RES = None


def _get_prog(a_scales):
    # a_scales are baked into the program as immediates -> part of the key
    akey = tuple(tuple(row) for row in a_scales)
    key = ("prog", N_LAYERS, DEBUG_DUMP_H, akey)
    if key not in _prog_cache:
        nc = _build_program(a_scales, N_LAYERS, DEBUG_DUMP_H)
        _split_excess_waits(nc)
        _prog_cache[key] = nc
    return _prog_cache[key]


def _run(nc, per_core):
    global LAST_EXEC_NS, LAST_RES
    res = run_bass_kernel_spmd(nc, per_core, core_ids=list(range(NCORES)),
                               trace=TRACE)
    LAST_EXEC_NS = res.exec_time_ns
    LAST_RES = res
    return res


def kernel(**inputs):
    per_core, a_scales = _host_prep(inputs)
    nc = _get_prog(a_scales)
    res = _run(nc, per_core)
    out = np.empty((B, L, VOCAB), np.float32)
    for c in range(NCORES):
        b, s = divmod(c, 4)
        out[b, LSH * s: LSH * (s + 1), :] = \
            res.results[c]["logits"][:, LSH * s: LSH * (s + 1)].T
    if DEBUG_DUMP_H:
        kernel.last_h = [res.results[c].get("hdump") for c in range(NCORES)]
        kernel.last_res = res
    return out
